# revision 1
# baseline (speedup 1.0000x reference)
"""MultiHeadAttention Trainium2 kernel (8 NeuronCores, SPMD).

Sharding: data-parallel over batch (B=2), tensor-parallel over heads
(16 heads -> 4 per core).  Core c handles batch b=c//4, head group
g=c%4 (heads 4g..4g+3).  Wq/Wk/Wv are split column-wise, Wo row-wise;
the per-core Wo partial outputs are summed on the host (replaces the
all-reduce).

Device dataflow per core (bf16 matmuls, f32 PSUM accumulation):
  qT = Wq_g^T x^T   [256, 2048]   (heads on partitions, dh=64 each)
  kT = Wk_g^T x^T   [256, 2048]
  v  = x Wv_g       [2048, 256] stored interleaved with a ones column
                    per head: vaug[st] = [vA|1|vB|1|vC|1|vD|1]
  per (s_q chunk of 512, head pair):
    logitsT[s_k, s_q] = kT^T qT / 8       (two heads packed in PE row
                                           groups, K=64 each)
    p = exp(logitsT)  on ScalarE, scale=1/8 fused, bf16 out
    accT[65, s_q] += vaug_h^T p           (row 64 = softmax denominator)
    outcatT[h] = accT[0:64] * bcast(1/accT[64])   (deferred softmax norm)
  partial = outcatT^T Wo_g  -> DRAM bf16 (summed in f32 on the host)

The kernel is organized as one flat software pipeline: the exp stream
on ScalarE is the pacer (~1.15us per (c,pr,st) step); everything else
(projections, Wo, output DMA) is deadline-scheduled into the PE slack
under it.  Inputs are loaded in 512-column chunks (one 3D DMA
descriptor per chunk) so the first exp fires after ~4.5MB of DMA
instead of the full 14.7MB.
"""

import itertools
import sys

import numpy as np

sys.path.insert(0, "/opt/trn_rl_repo")

import ml_dtypes  # noqa: E402

import concourse.bass as bass  # noqa: E402
import concourse.mybir as mybir  # noqa: E402
import concourse.tile as tile  # noqa: E402
from concourse import bacc  # noqa: E402
from concourse.bass import ts  # noqa: E402
from concourse.bass_utils import run_bass_kernel_spmd  # noqa: E402

S = 2048  # sequence length (S * X)
D = 1024  # model dim
H = 16  # total heads
HL = 4  # heads per core
DH = 64  # head dim
DQ = HL * DH  # per-core projection width = 256
NK = D // 128  # K tiles for projections = 8
NST = S // 128  # s_k tiles = 16
NCH = S // 512  # s_q chunks = 4
LAG = 5  # front-to-accumulate pipeline distance (runahead for stalls)

BF16 = mybir.dt.bfloat16
F32 = mybir.dt.float32

TRACE = False
LAST_RESULTS = None

_BUILT = None


def _emit(ctx, tc, io):
    nc = tc.nc
    xq, xk, xv = io["xqT"], io["xkT"], io["xvT"]
    wq, wk, wv, wo = io["wq"], io["wk"], io["wv"], io["wo"]
    bq, bk, bv = io["bq"], io["bk"], io["bv"]
    out = io["out"]

    consts = ctx.enter_context(tc.tile_pool(name="consts", bufs=1))
    xin = ctx.enter_context(tc.tile_pool(name="xin", bufs=1))
    qk = ctx.enter_context(tc.tile_pool(name="qk", bufs=1))
    ptiles = ctx.enter_context(tc.tile_pool(name="ptiles", bufs=7))
    norm = ctx.enter_context(tc.tile_pool(name="norm", bufs=3))
    osb_pool = ctx.enter_context(tc.tile_pool(name="osb", bufs=4))
    psum_mm = ctx.enter_context(tc.tile_pool(name="psum_mm", bufs=4, space="PSUM"))
    psum_lg = ctx.enter_context(tc.tile_pool(name="psum_lg", bufs=2, space="PSUM"))

    # x and W live as single 3D tiles: [128, k_tile, cols].  One DMA
    # descriptor loads a 512-column chunk of all 8 k-tiles at once (the
    # per-dma_start enqueue cost on the issuing engine is ~650ns, so
    # descriptor count is what paces the input stream).
    wq_all = consts.tile([128, NK, DQ], BF16, tag="wq", name="wq_all")
    wk_all = consts.tile([128, NK, DQ], BF16, tag="wk", name="wk_all")
    wv_all = consts.tile([128, NK, DQ], BF16, tag="wv", name="wv_all")
    wo_all = consts.tile([128, 2, D], BF16, tag="wo", name="wo_all")
    xq_all = xin.tile([128, NK, S], BF16, tag="xq", name="xq_all")
    xk_all = xin.tile([128, NK, S], BF16, tag="xk", name="xk_all")
    xv_all = xin.tile([128, NK, S], BF16, tag="xv", name="xv_all")
    # bq/bk as [128, 2] per-partition scalars (col j = dq 128j..128j+127)
    bq_sb = consts.tile([128, 2], F32, tag="bq", name="bq_sb")
    bk_sb = consts.tile([128, 2], F32, tag="bk", name="bk_sb")
    bv_sb = consts.tile([128, DQ], F32, tag="bv", name="bv_sb")

    # PE warmup: dummy back-to-back matmuls at t=0, while the PE would
    # otherwise sit idle waiting for input DMA.  The PE clock gate (HAM)
    # defaults to 4/8 throttle (1.2 GHz) and only releases after ~3.4us
    # of sustained activity; without this the whole DMA-paced front runs
    # at half clock.
    wu_sb = consts.tile([128, 512], BF16, tag="wu", name="wu_sb")
    nc.vector.memset(wu_sb[:], 1.0)
    wu_ps = psum_lg.tile([128, 512], F32, tag="lg", name="lg")

    for _ in range(26):
        nc.tensor.matmul(wu_ps[:], wu_sb[:, 0:128], wu_sb[:], start=True, stop=True)

    # ---- input DMA: weights, then 512-col chunks in just-in-time order,
    # alternating the two descriptor queues.
    nc.gpsimd.dma_start(
        out=bk_sb[:], in_=bass.AP(tensor=bk.tensor, offset=bk.offset, ap=[[1, 128], [128, 2]])
    )
    nc.gpsimd.dma_start(
        out=bq_sb[:], in_=bass.AP(tensor=bq.tensor, offset=bq.offset, ap=[[1, 128], [128, 2]])
    )
    nc.gpsimd.dma_start(
        out=bv_sb[:], in_=bass.AP(tensor=bv.tensor, offset=bv.offset, ap=[[0, 128], [1, DQ]])
    )
    qsel = itertools.cycle([nc.sync, nc.gpsimd])

    def dma_w(dst, src, nkt, cols):
        next(qsel).dma_start(
            out=dst[:],
            in_=bass.AP(
                tensor=src.tensor,
                offset=src.offset,
                ap=[[cols, 128], [128 * cols, nkt], [1, cols]],
            ),
        )

    def dma_chunk(dst, src, c):
        next(qsel).dma_start(
            out=dst[:, :, ts(c, 512)],
            in_=bass.AP(
                tensor=src.tensor,
                offset=src.offset + c * 512,
                ap=[[S, 128], [128 * S, NK], [1, 512]],
            ),
        )

    dma_w(wk_all, wk, NK, DQ)
    dma_w(wq_all, wq, NK, DQ)
    dma_w(wv_all, wv, NK, DQ)
    dma_chunk(xk_all, xk, 0)
    dma_chunk(xv_all, xv, 0)
    dma_chunk(xq_all, xq, 0)
    for c in range(1, NCH):
        dma_chunk(xk_all, xk, c)
        dma_chunk(xv_all, xv, c)
    dma_chunk(xq_all, xq, 1)
    dma_w(wo_all, wo, 2, D)
    dma_chunk(xq_all, xq, 2)
    dma_chunk(xq_all, xq, 3)

    # ---- projections: qT, kT = [256, 2048] as 2 tiles of [128, 2048] ----
    qT = [qk.tile([128, S], BF16, tag=f"qT{m}", name=f"qT{m}") for m in range(2)]
    kT = [qk.tile([128, S], BF16, tag=f"kT{m}", name=f"kT{m}") for m in range(2)]

    def qk_half(w_all, dst, b_sb, m, c, on_act, state, phase):
        """Half of a q/k projection PSUM group (k 0-3 or 4-7).

        Split so each injected PE lump is ~0.9us: the front can only run
        ~2 steps (~1.4us of slack cushion) ahead of the exp stream
        (psum_lg bufs), so a full 8-matmul group injected at once stalls
        the exp feed by ~1.5us.  The PSUM group stays open between the
        halves; unrelated matmuls interleave freely.
        """
        x_all = xq_all if dst is qT else xk_all
        if phase == 0:
            state["ps"] = psum_mm.tile([128, 512], F32, tag="mm", name="mm")
        ps = state["ps"]
        for k in range(4 * phase, 4 * phase + 4):
            nc.tensor.matmul(
                ps[:],
                w_all[:, k, ts(m, 128)],
                x_all[:, k, ts(c, 512)],
                start=(k == 0),
                stop=(k == NK - 1),
            )
        if phase == 1:
            if on_act:  # prefix phase: ScalarE is idle there
                nc.scalar.add(dst[m][:, ts(c, 512)], ps[:], b_sb[:, m : m + 1])
            else:  # injected into attention: keep ScalarE free for exp
                nc.vector.tensor_scalar_add(dst[m][:, ts(c, 512)], ps[:], b_sb[:, m : m + 1])

    def qk_group(w_all, dst, b_sb, m, c, on_act):
        state = {}
        qk_half(w_all, dst, b_sb, m, c, on_act, state, 0)
        qk_half(w_all, dst, b_sb, m, c, on_act, state, 1)

    # vaug[st] = [vA|1|vB|1|vC|1|vD|1]  [128, 260]
    vaug = [qk.tile([128, HL * (DH + 1)], BF16, tag=f"vaug{st}", name=f"vaug{st}") for st in range(NST)]

    def v_piece(j):
        # 2 st tiles per piece, k outermost
        sts = (2 * j, 2 * j + 1)
        pss = {st: psum_mm.tile([128, DQ], F32, tag="mm", name="mm") for st in sts}
        for k in range(NK):
            for st in sts:
                nc.tensor.matmul(
                    pss[st][:],
                    xv_all[:, k, ts(st, 128)],
                    wv_all[:, k, :],
                    start=(k == 0),
                    stop=(k == NK - 1),
                )
        for st in sts:
            nc.vector.memset(vaug[st][:], 1.0)
            for h in range(HL):
                nc.vector.tensor_add(
                    vaug[st][:, h * 65 : h * 65 + 64],
                    pss[st][:, ts(h, DH)],
                    bv_sb[:, ts(h, DH)],
                )

    octT = [qk.tile([128, S], BF16, tag=f"octT{m}", name=f"octT{m}") for m in range(2)]
    store_q = itertools.cycle([nc.sync, nc.gpsimd])
    tail_cp = itertools.cycle([True, False])  # alternate scalar/vector copies

    def wo_group(c, smt, ncho, on_act=False):
        row = c * 512 + smt * 128
        ps = psum_mm.tile([128, 512], F32, tag="mm", name="mm")
        for k in range(2):
            nc.tensor.matmul(
                ps[:],
                octT[k][:, row : row + 128],
                wo_all[:, k, ts(ncho, 512)],
                start=(k == 0),
                stop=(k == 1),
            )
        osb = osb_pool.tile([128, 512], BF16, tag="osb", name="osb")
        if on_act and next(tail_cp):
            # tail drain: split copies between the idle ScalarE and DVE
            nc.scalar.copy(osb[:], ps[:])
        else:
            nc.vector.tensor_copy(osb[:], ps[:])
        next(store_q).dma_start(out[row : row + 128, ts(ncho, 512)], osb[:])

    # ---- prefix: just enough for the first exp + the first AV steps:
    # kT chunk 0 / qT chunk 0 of the pr0 pair, vaug st 0-3.  Everything
    # else is deadline-scheduled into the attention pipeline below.
    qk_group(wk_all, kT, bk_sb, 0, 0, on_act=True)
    qk_group(wq_all, qT, bq_sb, 0, 0, on_act=True)
    v_piece(0)
    v_piece(1)

    # ---- attention: one flat software pipeline over all (c, pr, st)
    # steps, ScalarE-exp paced.  Projection/Wo work is injected by
    # deadline (the step by which its output is consumed), in ~0.9us
    # half-group lumps the front cushion can absorb.
    steps = [(c, pr, st) for c in range(NCH) for pr in range(2) for st in range(NST)]

    jobs = []  # (deadline_step, fn)

    for j in range(2, 8):  # vaug st 2j,2j+1 consumed at acc step 2j
        jobs.append((2 * j + 2, lambda jj=j: v_piece(jj)))
    for c in range(1, NCH):  # kT m0 chunk c consumed at front step 4c
        jobs.append((4 * c - 2, lambda cc=c: qk_group(wk_all, kT, bk_sb, 0, cc, False)))
    for c in range(NCH):  # kT m1 chunk c consumed at front step 16+4c
        jobs.append((14 + 4 * c, lambda cc=c: qk_group(wk_all, kT, bk_sb, 1, cc, False)))
    jobs.append((14, lambda: qk_group(wq_all, qT, bq_sb, 1, 0, False)))
    for c in range(1, NCH):  # qT m chunk c consumed from front step 32c+16m
        jobs.append((32 * c - 2, lambda cc=c: qk_group(wq_all, qT, bq_sb, 0, cc, False)))
        jobs.append((32 * c + 14, lambda cc=c: qk_group(wq_all, qT, bq_sb, 1, cc, False)))
    jobs.sort(key=lambda t: t[0])

    wo_q = []
    acc_map = {}
    p_map = {}

    def emit_front(c, pr, st):
        lg = psum_lg.tile([128, 1024], F32, tag="lg", name="lg")
        for hh in range(2):
            nc.tensor.matmul(
                lg[:, ts(hh, 512)],
                kT[pr][ts(hh, 64), ts(st, 128)],
                qT[pr][ts(hh, 64), ts(c, 512)],
                start=True,
                stop=True,
            )
        p = ptiles.tile([128, 1024], BF16, tag="p", name="p")
        nc.scalar.activation(p[:], lg[:], mybir.ActivationFunctionType.Exp, scale=0.125)
        p_map[(c, pr, st)] = p

    def emit_acc(c, pr, st):
        if st == 0:
            acc_map[(c, pr)] = [
                psum_mm.tile([65, 512], F32, tag="mm", name="mm") for _ in range(2)
            ]
        acc = acc_map[(c, pr)]
        pp = p_map.pop((c, pr, st))
        for hh in range(2):
            h = 2 * pr + hh
            nc.tensor.matmul(
                acc[hh][:],
                vaug[st][:, h * 65 : h * 65 + 65],
                pp[:, ts(hh, 512)],
                start=(st == 0),
                stop=(st == NST - 1),
            )
        if st == NST - 1:
            # normalize: octT[pr][64*hh, chunk c] = acc[0:64] / acc[64].
            # Reciprocal on the [1,512] denominator row, gpsimd broadcast
            # to 64 partitions, one DVE mul straight out of acc PSUM.
            for hh in range(2):
                den = norm.tile([1, 512], F32, tag="den", name="den")
                nc.vector.tensor_copy(den[:], acc[hh][64:65, :])
                rden = norm.tile([1, 512], F32, tag="rden", name="rden")
                nc.vector.reciprocal_approx_fast(rden[:], den[:])
                bc = norm.tile([64, 512], F32, tag="bcs", name="bcs")
                nc.gpsimd.partition_broadcast(bc[:], rden[:])
                nc.vector.tensor_mul(octT[pr][ts(hh, 64), ts(c, 512)], acc[hh][0:64, :], bc[:])
            del acc_map[(c, pr)]
            if pr == 1:
                kwargs = {"on_act": True} if c == NCH - 1 else {}
                wo_q.extend(
                    (lambda cc=c, smt=smt, ncho=ncho, kw=kwargs: wo_group(cc, smt, ncho, **kw))
                    for smt in range(4)
                    for ncho in range(2)
                )

    ji = 0
    for i, s in enumerate(steps):
        while ji < len(jobs) and jobs[ji][0] <= i:
            jobs[ji][1]()
            ji += 1
        emit_front(*s)
        if i >= LAG:
            emit_acc(*steps[i - LAG])
        c, pr, st = s
        # filler slot at odd steps: pull forward a near-due job, else
        # drain the Wo backlog
        if st % 2 == 1 and not (pr == 0 and st == 1):
            if ji < len(jobs) and jobs[ji][0] <= i + 4:
                jobs[ji][1]()
                ji += 1
            elif wo_q:
                wo_q.pop(0)()

    for i in range(len(steps) - LAG, len(steps)):
        emit_acc(*steps[i])
    while ji < len(jobs):
        jobs[ji][1]()
        ji += 1

    # Keep the PE clocked (HAM) while the last chunk's normalize chain
    # completes on DVE/gpsimd (~4us: the first Wo group's PSUM slot waits
    # on the acc-releasing muls); otherwise the final Wo groups run at
    # 4/8 throttle.
    for _ in range(8):
        nc.tensor.matmul(wu_ps[:], wu_sb[:, 0:128], wu_sb[:], start=True, stop=True)

    for g in wo_q:
        g()


def _build():
    global _BUILT
    if _BUILT is not None:
        return _BUILT
    nc = bacc.Bacc(
        "TRN2",
        target_bir_lowering=False,
        debug=False,
        enable_asserts=False,
        num_devices=8,
    )
    io = {}
    io["xqT"] = nc.dram_tensor("xqT", [D, S], BF16, kind="ExternalInput").ap()
    io["xkT"] = nc.dram_tensor("xkT", [D, S], BF16, kind="ExternalInput").ap()
    io["xvT"] = nc.dram_tensor("xvT", [D, S], BF16, kind="ExternalInput").ap()
    io["wq"] = nc.dram_tensor("wq", [D, DQ], BF16, kind="ExternalInput").ap()
    io["wk"] = nc.dram_tensor("wk", [D, DQ], BF16, kind="ExternalInput").ap()
    io["wv"] = nc.dram_tensor("wv", [D, DQ], BF16, kind="ExternalInput").ap()
    io["wo"] = nc.dram_tensor("wo", [DQ, D], BF16, kind="ExternalInput").ap()
    io["bq"] = nc.dram_tensor("bq", [DQ], F32, kind="ExternalInput").ap()
    io["bk"] = nc.dram_tensor("bk", [DQ], F32, kind="ExternalInput").ap()
    io["bv"] = nc.dram_tensor("bv", [DQ], F32, kind="ExternalInput").ap()
    io["out"] = nc.dram_tensor("out", [S, D], BF16, kind="ExternalOutput").ap()
    from contextlib import ExitStack

    with tile.TileContext(nc) as tc, ExitStack() as ctx:
        _emit(ctx, tc, io)
    nc.compile()
    _BUILT = nc
    return nc


def kernel(**inputs):
    global LAST_RESULTS
    bf16 = ml_dtypes.bfloat16
    query = np.asarray(inputs["query"], np.float32).reshape(2, S, D)
    key = np.asarray(inputs["key"], np.float32).reshape(2, S, D)
    value = np.asarray(inputs["value"], np.float32).reshape(2, S, D)
    Wq = np.asarray(inputs["Wq"], np.float32)
    Wk = np.asarray(inputs["Wk"], np.float32)
    Wv = np.asarray(inputs["Wv"], np.float32)
    Wo = np.asarray(inputs["Wo"], np.float32)
    bq = np.asarray(inputs["bq"], np.float32)
    bk = np.asarray(inputs["bk"], np.float32)
    bv = np.asarray(inputs["bv"], np.float32)
    bo = np.asarray(inputs["bo"], np.float32)

    xT = {}
    for b in range(2):
        xT[("q", b)] = np.ascontiguousarray(query[b].T).astype(bf16)
        xT[("k", b)] = np.ascontiguousarray(key[b].T).astype(bf16)
        xT[("v", b)] = np.ascontiguousarray(value[b].T).astype(bf16)

    in_maps = []
    for c in range(8):
        b, g = c // 4, c % 4
        sl = slice(g * DQ, (g + 1) * DQ)
        in_maps.append(
            {
                "xqT": xT[("q", b)],
                "xkT": xT[("k", b)],
                "xvT": xT[("v", b)],
                "wq": np.ascontiguousarray(Wq[:, sl]).astype(bf16),
                "wk": np.ascontiguousarray(Wk[:, sl]).astype(bf16),
                "wv": np.ascontiguousarray(Wv[:, sl]).astype(bf16),
                "wo": np.ascontiguousarray(Wo[sl, :]).astype(bf16),
                "bq": np.ascontiguousarray(bq[sl]),
                "bk": np.ascontiguousarray(bk[sl]),
                "bv": np.ascontiguousarray(bv[sl]),
            }
        )

    nc = _build()
    res = run_bass_kernel_spmd(
        nc, in_maps, core_ids=list(range(8)), trace=TRACE
    )
    LAST_RESULTS = res

    full = np.zeros((2, S, D), np.float32)
    for c in range(8):
        full[c // 4] += res.results[c]["out"].astype(np.float32)
    full += bo[None, None, :]
    return full.reshape(2, S, 1, D)

